# revision 1
# baseline (speedup 1.0000x reference)
"""DCGRU cell on 8 Trainium2 NeuronCores.

Strategy (dst-sharded graph partitioning):
  - Nodes are sharded into 8 contiguous ranges (one per core). Edges are
    assigned to the core owning dst; within a core, dst nodes are processed
    in blocks of 128, grouped into super-blocks of SB_BLOCKS for gathering.
  - Because dma_gather takes int16 indices, the gather tables are split in
    two halves (src < N/2 and src >= N/2); per (block, half) the edges are
    padded to groups of 128 (group counts uniform across cores so a single
    SPMD instruction stream works). One dma_gather covers a whole
    (super-block, half) run of groups.
  - Pass 1 gathers x1 = [feat, state] rows (bf16, 256B) and segment-sums
    them via one-hot matmuls into PSUM (transposed layout), then
    zr = sigmoid(aggT.T @ Wzr + bzr); rs = r * state.
  - y2 = feat @ Wc_top + rs @ Wc_bot is computed per block (bf16) and
    AllGathered across cores (6.4MB). Pass 2 gathers 256B rows each holding
    a PAIR of adjacent nodes' y2 vectors (index src_pos//2); a single
    [128, 256] "paired one-hot" per group (iota vs dst_local + 128*parity)
    feeds two matmuls that select the correct half, giving
    c = tanh(agg + bc); new_state = z*state + (1-z)*c.
"""

import numpy as np

import concourse.bass as bass
import concourse.bacc as bacc
import concourse.mybir as mybir
import concourse.tile as tile
from concourse.bass_utils import run_bass_kernel_spmd
from concourse.library_config import mlp
from concourse.masks import make_identity

N_NODES = 50000
N_EDGES = 640000
HID = 64
N_CORES = 8
BLK = 128           # dst nodes per block (= PSUM partition dim)
SB_BLOCKS = 1       # dst blocks per super-block (gather instruction scope)
MAX_G_CHUNK = 12    # cap on groups per dma_gather instruction (pass 1)
MAX_GC_CHUNK = 20   # cap on groups per dma_gather instruction (pass 2)

F32 = mybir.dt.float32
BF16 = mybir.dt.bfloat16
I16 = mybir.dt.int16


def _prep_edges(dst, src, edge_weight, n_nodes, n_cores):
    """Partition edges by dst core/block and src half; build per-core arrays.

    Group order: for each super-block, all lo-half groups of its blocks,
    then all hi-half groups.

    Returns (idx16, dst_t, w_t, plan) where plan is a dict with
      chunks:   list of (g0, g1, cls) gather chunk spans
      blk_of_g: block id of each group
      first_g, last_g: per block, first/last group id
      nblk, ngroups
    """
    shard = n_nodes // n_cores
    nblk = (shard + BLK - 1) // BLK
    split = n_nodes // 2
    e = len(dst)

    dsts = dst.astype(np.int64)
    srcs = src.astype(np.int64)
    owner = dsts // shard
    local = dsts - owner * shard

    # Balance in-degree across blocks: per core, deal nodes (sorted by
    # in-degree, desc) round-robin over blocks. pos[core, orig_local] is the
    # node's new row; node tables / shard rows / outputs use this order.
    deg = np.zeros(n_nodes, np.int64)
    np.add.at(deg, dsts, 1)
    pos = np.empty((n_cores, shard), np.int64)
    blk_fill = np.empty(nblk, np.int64)
    cap = np.full(nblk, BLK, np.int64)
    cap[nblk - 1] = shard - (nblk - 1) * BLK
    for p in range(n_cores):
        nodes = np.argsort(-deg[p * shard : (p + 1) * shard], kind="stable")
        blk_fill[:] = 0
        bi = 0
        for n in nodes:
            while blk_fill[bi % nblk] >= cap[bi % nblk]:
                bi += 1
            b = bi % nblk
            pos[p, n] = b * BLK + blk_fill[b]
            blk_fill[b] += 1
            bi += 1
    newloc = pos[owner, local]
    b_of = newloc // BLK
    local = newloc
    cls = (srcs >= split).astype(np.int64)

    cnt = np.zeros((n_cores, nblk, 2), np.int64)
    np.add.at(cnt, (owner, b_of, cls), 1)
    gpbc = -(-cnt.max(axis=0) // BLK)  # [nblk, 2] groups per (block, half)
    # ensure every block has at least one group so its PSUM accum is defined
    empty = gpbc.sum(axis=1) == 0
    gpbc[empty, 0] = 1

    # unit order: (super-block, class, block)
    unit_order = []
    for sb0 in range(0, nblk, SB_BLOCKS):
        sbb = range(sb0, min(sb0 + SB_BLOCKS, nblk))
        for c in range(2):
            for b in sbb:
                unit_order.append((b, c))
    unit_sizes = np.array([gpbc[b, c] for b, c in unit_order], np.int64)
    unit_off = np.concatenate([[0], np.cumsum(unit_sizes)])
    ngroups = int(unit_off[-1])
    unit_idx = {bc: i for i, bc in enumerate(unit_order)}

    # per-group block id and per-block first/last group
    blk_of_g = np.zeros(ngroups, np.int64)
    for i, (b, c) in enumerate(unit_order):
        blk_of_g[unit_off[i] : unit_off[i + 1]] = b
    first_g = np.full(nblk, -1, np.int64)
    last_g = np.full(nblk, -1, np.int64)
    for g in range(ngroups):
        b = blk_of_g[g]
        if first_g[b] < 0:
            first_g[b] = g
        last_g[b] = g

    # gather chunks: contiguous same-class unit runs within a super-block
    chunks = []
    i = 0
    while i < len(unit_order):
        c = unit_order[i][1]
        sb = unit_order[i][0] // SB_BLOCKS
        j = i
        while (
            j < len(unit_order)
            and unit_order[j][1] == c
            and unit_order[j][0] // SB_BLOCKS == sb
        ):
            j += 1
        g0, g1 = int(unit_off[i]), int(unit_off[j])
        for s in range(g0, g1, MAX_G_CHUNK):
            if s < g1:
                chunks.append((s, min(s + MAX_G_CHUNK, g1), c))
        i = j

    # slot assignment
    ukey = np.array([unit_idx[(b, c)] for b, c in zip(b_of, cls)], np.int64)
    order = np.argsort(ukey, kind="stable")
    ukey_s = ukey[order]
    owner_s = owner[order]
    srcs_s = srcs[order]
    cls_s = cls[order]
    dloc_s = (local % BLK)[order].astype(np.float32)
    ws_s = edge_weight.astype(np.float32)[order]
    src_pos = pos[srcs_s // shard, srcs_s % shard] + (srcs_s // shard) * shard
    src_local = src_pos - cls_s * split

    # rank within (core, unit)
    ck = owner_s * len(unit_order) + ukey_s
    order2 = np.argsort(ck, kind="stable")
    ck = ck[order2]
    owner_s = owner_s[order2]
    src_local = src_local[order2]
    ukey_s = ukey_s[order2]
    dloc_s = dloc_s[order2]
    ws_s = ws_s[order2]
    cls_sorted2 = cls_s[order2]
    bucket_start = np.searchsorted(ck, np.arange(n_cores * len(unit_order)))
    rank = np.arange(e) - bucket_start[ck]
    g_global = unit_off[ukey_s] + rank // BLK
    lane = rank % BLK

    # recover permuted global src position for pass-2 pair indexing
    src_posg = src_local + cls_sorted2 * split
    par = (src_posg % 2).astype(np.float32)
    pair_idx = src_posg // 2

    idx16 = np.zeros((n_cores, 16, 8 * ngroups), np.int16)
    idx16p = np.zeros((n_cores, 16, 8 * ngroups), np.int16)
    dst_t = np.zeros((n_cores, BLK, ngroups), np.float32)
    dstp_t = np.zeros((n_cores, BLK, ngroups), np.float32)
    w_t = np.zeros((n_cores, BLK, ngroups), np.float32)
    idx16[owner_s, lane % 16, 8 * g_global + lane // 16] = src_local.astype(np.int16)
    idx16p[owner_s, lane % 16, 8 * g_global + lane // 16] = pair_idx.astype(np.int16)
    dst_t[owner_s, lane, g_global] = dloc_s
    dstp_t[owner_s, lane, g_global] = dloc_s + BLK * par
    w_t[owner_s, lane, g_global] = ws_s
    idx16 = np.tile(idx16, (1, 8, 1))
    idx16p = np.tile(idx16p, (1, 8, 1))

    # phase-C chunks: pass 2 has a single gather table, so each block's
    # whole group span is one chunk (split only by the SBUF tile cap)
    chunks_c = []
    for b in range(nblk):
        ga, gb = int(first_g[b]), int(last_g[b]) + 1
        for s in range(ga, gb, MAX_GC_CHUNK):
            chunks_c.append((s, min(s + MAX_GC_CHUNK, gb), 0))

    plan = {
        "chunks": chunks,
        "chunks_c": chunks_c,
        "blk_of_g": [int(x) for x in blk_of_g],
        "first_g": [int(x) for x in first_g],
        "last_g": [int(x) for x in last_g],
        "nblk": nblk,
        "ngroups": ngroups,
        "pos": pos,
    }
    return idx16, idx16p, dst_t, dstp_t, w_t, plan


def _build(n_nodes, hid, plan, n_cores, n_queues=4):
    """Build the SPMD Bass program from the edge plan."""
    shard = n_nodes // n_cores
    nblk = plan["nblk"]
    ngroups = plan["ngroups"]
    chunks = plan["chunks"]
    blk_of_g = plan["blk_of_g"]
    first_g = plan["first_g"]
    last_g = plan["last_g"]
    split = n_nodes // 2
    h2 = 2 * hid

    nc = bacc.Bacc(None, num_devices=n_cores, num_swdge_queues=n_queues)

    x1b = nc.dram_tensor("x1b", [n_nodes, h2], BF16, kind="ExternalInput")
    feat_s = nc.dram_tensor("feat_s", [shard, hid], F32, kind="ExternalInput")
    state_s = nc.dram_tensor("state_s", [shard, hid], F32, kind="ExternalInput")
    idx16_d = nc.dram_tensor("idx16", [BLK, 8 * ngroups], I16, kind="ExternalInput")
    idx16p_d = nc.dram_tensor("idx16p", [BLK, 8 * ngroups], I16, kind="ExternalInput")
    dst_d = nc.dram_tensor("dst_t", [BLK, ngroups], F32, kind="ExternalInput")
    dstp_d = nc.dram_tensor("dstp_t", [BLK, ngroups], F32, kind="ExternalInput")
    w_d = nc.dram_tensor("w_t", [BLK, ngroups], F32, kind="ExternalInput")
    wzr = nc.dram_tensor("wzr", [h2, h2], F32, kind="ExternalInput")
    bzr = nc.dram_tensor("bzr", [1, h2], F32, kind="ExternalInput")
    wc = nc.dram_tensor("wc", [h2, hid], F32, kind="ExternalInput")
    bc = nc.dram_tensor("bc", [1, hid], F32, kind="ExternalInput")
    out = nc.dram_tensor("out", [shard, hid], F32, kind="ExternalOutput")

    y2_shard = nc.dram_tensor("y2_shard", [shard, hid], BF16, kind="Internal")
    y2_full = nc.dram_tensor(
        "y2_full", [n_nodes, hid], BF16, kind="Internal", addr_space="Shared"
    )

    qn = [0]

    def next_q():
        q = qn[0]
        qn[0] = (qn[0] + 1) % n_queues
        return q

    def rows_of(b):
        return BLK if b < nblk - 1 else shard - (nblk - 1) * BLK

    with tile.TileContext(nc) as tc:
        with (
            tc.tile_pool(name="const", bufs=1) as const_pool,
            tc.tile_pool(name="store", bufs=1) as store_pool,
            tc.tile_pool(name="msg", bufs=4) as msg_pool,
            tc.tile_pool(name="oh", bufs=6) as oh_pool,
            tc.tile_pool(name="blk", bufs=5) as blk_pool,
            tc.tile_pool(name="agg_ps", bufs=5, space="PSUM") as agg_psum,
            tc.tile_pool(name="mm_ps", bufs=3, space="PSUM") as mm_psum,
        ):
            nc.gpsimd.load_library(mlp)
            # ---- constants ----
            iota_i = const_pool.tile([BLK, BLK], mybir.dt.int32)
            nc.gpsimd.iota(iota_i[:], pattern=[[1, BLK]], base=0, channel_multiplier=0)
            iota_f = const_pool.tile([BLK, BLK], F32)
            nc.vector.tensor_copy(iota_f[:], iota_i[:])
            iota_h = const_pool.tile([BLK, BLK], BF16)
            nc.vector.tensor_copy(iota_h[:], iota_i[:])
            iota2_i = const_pool.tile([BLK, 2 * BLK], mybir.dt.int32)
            nc.gpsimd.iota(
                iota2_i[:], pattern=[[1, 2 * BLK]], base=0, channel_multiplier=0
            )
            iota2_h = const_pool.tile([BLK, 2 * BLK], BF16)
            nc.vector.tensor_copy(iota2_h[:], iota2_i[:])
            identity = const_pool.tile([BLK, BLK], F32)
            make_identity(nc, identity[:])
            ones1 = const_pool.tile([1, BLK], F32)
            nc.vector.memset(ones1[:], 1.0)
            wzr_sb = const_pool.tile([h2, h2], F32)
            nc.sync.dma_start(out=wzr_sb[:], in_=wzr[:, :])
            bzr_sb = const_pool.tile([1, h2], F32)
            nc.sync.dma_start(out=bzr_sb[:], in_=bzr[:, :])
            wctop_sb = const_pool.tile([hid, hid], F32)
            nc.sync.dma_start(out=wctop_sb[:], in_=wc[0:hid, :])
            wcbot_sb = const_pool.tile([hid, hid], F32)
            nc.sync.dma_start(out=wcbot_sb[:], in_=wc[hid:h2, :])
            bc_sb = const_pool.tile([1, hid], F32)
            nc.sync.dma_start(out=bc_sb[:], in_=bc[:, :])

            # ---- persistent stores (indices/weights loaded once) ----
            idx16_sb = store_pool.tile([BLK, 8 * ngroups], I16)
            nc.sync.dma_start(out=idx16_sb[:], in_=idx16_d[:, :])
            idx16p_sb = store_pool.tile([BLK, 8 * ngroups], I16)
            nc.sync.dma_start(out=idx16p_sb[:], in_=idx16p_d[:, :])
            dst_sb = store_pool.tile([BLK, ngroups], F32)
            nc.sync.dma_start(out=dst_sb[:], in_=dst_d[:, :])
            w_sb = store_pool.tile([BLK, ngroups], F32)
            nc.sync.dma_start(out=w_sb[:], in_=w_d[:, :])
            dstp_sb = store_pool.tile([BLK, ngroups], F32)
            nc.sync.dma_start(out=dstp_sb[:], in_=dstp_d[:, :])
            z_store = store_pool.tile([BLK, nblk * hid], F32)
            st_store = store_pool.tile([BLK, nblk * hid], F32)
            nc.vector.memset(z_store[:], 0.0)
            nc.vector.memset(st_store[:], 0.0)

            # ============== Phase A: pass-1 aggregation + y2 ===============
            psum_of = {}

            def tail_a(b):
                """Post-aggregation per-block work for pass 1."""
                R = rows_of(b)
                aggT_sb = blk_pool.tile([h2, BLK], F32, tag="aggT")
                nc.vector.tensor_copy(aggT_sb[:], psum_of.pop(b)[:])
                zr_ps = mm_psum.tile([BLK, h2], F32, tag="mm")
                nc.tensor.matmul(
                    zr_ps[:], lhsT=aggT_sb[:], rhs=wzr_sb[:], start=True, stop=False
                )
                nc.tensor.matmul(
                    zr_ps[:], lhsT=ones1[:], rhs=bzr_sb[:], start=False, stop=True
                )
                zr_sb = blk_pool.tile([BLK, h2], F32, tag="zr")
                nc.scalar.activation(
                    zr_sb[:], zr_ps[:], mybir.ActivationFunctionType.Sigmoid
                )
                nc.vector.tensor_copy(
                    z_store[:, b * hid : (b + 1) * hid], zr_sb[:, 0:hid]
                )
                nc.sync.dma_start(
                    out=st_store[:R, b * hid : b * hid + hid],
                    in_=state_s[b * BLK : b * BLK + R, :],
                )
                rs = blk_pool.tile([BLK, hid], F32, tag="rs")
                nc.vector.tensor_tensor(
                    out=rs[:],
                    in0=zr_sb[:, hid:h2],
                    in1=st_store[:, b * hid : (b + 1) * hid],
                    op=mybir.AluOpType.mult,
                )
                featb = blk_pool.tile([BLK, hid], F32, tag="featb")
                nc.vector.memset(featb[:], 0.0)
                nc.sync.dma_start(
                    out=featb[:R, :], in_=feat_s[b * BLK : b * BLK + R, :]
                )
                tp_f = mm_psum.tile([hid, BLK], F32, tag="mm")
                nc.tensor.transpose(out=tp_f[:], in_=featb[:], identity=identity[:])
                featT = blk_pool.tile([hid, BLK], F32, tag="featT")
                nc.vector.tensor_copy(featT[:], tp_f[:])
                tp_r = mm_psum.tile([hid, BLK], F32, tag="mm")
                nc.tensor.transpose(out=tp_r[:], in_=rs[:], identity=identity[:])
                rsT = blk_pool.tile([hid, BLK], F32, tag="rsT")
                nc.vector.tensor_copy(rsT[:], tp_r[:])
                y2_ps = mm_psum.tile([BLK, hid], F32, tag="mm")
                nc.tensor.matmul(
                    y2_ps[:], lhsT=featT[:], rhs=wctop_sb[:], start=True, stop=False
                )
                nc.tensor.matmul(
                    y2_ps[:], lhsT=rsT[:], rhs=wcbot_sb[:], start=False, stop=True
                )
                y2_sb = blk_pool.tile([BLK, hid], BF16, tag="y2")
                nc.vector.tensor_copy(y2_sb[:], y2_ps[:])
                nc.sync.dma_start(
                    out=y2_shard[b * BLK : b * BLK + R, :], in_=y2_sb[:R, :]
                )

            for g0, g1, c in chunks:
                kg = g1 - g0
                nidx = kg * BLK
                tbl = x1b[0:split, :] if c == 0 else x1b[split:n_nodes, :]
                msgs = msg_pool.tile([BLK, MAX_G_CHUNK * h2], BF16, tag="m1")
                out_ap = msgs[:, : kg * h2].rearrange("p (t w) -> p t w", w=h2)
                nc.gpsimd.dma_gather(
                    out_ap,
                    tbl,
                    idx16_sb[:, 8 * g0 : 8 * g1],
                    nidx,
                    nidx,
                    h2,
                    queue_num=next_q(),
                    single_packet=False,
                )
                for g in range(g0, g1):
                    b = blk_of_g[g]
                    if b not in psum_of:
                        psum_of[b] = agg_psum.tile([h2, BLK], F32, tag="agg", name=f"agga{b}")
                    oh = oh_pool.tile([BLK, BLK], BF16, tag="oh")
                    nc.vector.tensor_scalar(
                        out=oh[:],
                        in0=iota_h[:],
                        scalar1=dst_sb[:, g : g + 1],
                        scalar2=w_sb[:, g : g + 1],
                        op0=mybir.AluOpType.is_equal,
                        op1=mybir.AluOpType.mult,
                    )
                    gl = (g - g0) * h2
                    nc.tensor.matmul(
                        out=psum_of[b][:],
                        lhsT=msgs[:, gl : gl + h2],
                        rhs=oh[:],
                        start=(g == first_g[b]),
                        stop=(g == last_g[b]),
                    )
                    if g == last_g[b]:
                        tail_a(b)

            # ================= Phase B: AllGather y2 ========================
            nc.gpsimd.collective_compute(
                "AllGather",
                mybir.AluOpType.bypass,
                replica_groups=[list(range(n_cores))],
                ins=[y2_shard[:, :]],
                outs=[y2_full[:, :]],
            )

            # ============== Phase C: pass-2 aggregation + output ===========
            def tail_c(b):
                R = rows_of(b)
                psum_c = psum_of.pop(b)
                nc.tensor.matmul(
                    psum_c[:, :hid], lhsT=ones1[:], rhs=bc_sb[:], start=False, stop=True
                )
                c_sb = blk_pool.tile([BLK, hid], F32, tag="c")
                nc.scalar.activation(
                    c_sb[:], psum_c[:, :hid], mybir.ActivationFunctionType.Tanh
                )
                # new_state = c + z*(state - c)
                t1 = blk_pool.tile([BLK, hid], F32, tag="t1")
                nc.vector.tensor_tensor(
                    out=t1[:],
                    in0=st_store[:, b * hid : (b + 1) * hid],
                    in1=c_sb[:],
                    op=mybir.AluOpType.subtract,
                )
                t2 = blk_pool.tile([BLK, hid], F32, tag="t2")
                nc.vector.tensor_tensor(
                    out=t2[:],
                    in0=t1[:],
                    in1=z_store[:, b * hid : (b + 1) * hid],
                    op=mybir.AluOpType.mult,
                )
                ns = blk_pool.tile([BLK, hid], F32, tag="ns")
                nc.vector.tensor_tensor(
                    out=ns[:], in0=t2[:], in1=c_sb[:], op=mybir.AluOpType.add
                )
                nc.sync.dma_start(
                    out=out[b * BLK : b * BLK + R, :], in_=ns[:R, :]
                )

            y2_pairs = y2_full[:, :].rearrange("(n two) h -> n (two h)", two=2)
            for g0, g1, c in chunks:
                kg = g1 - g0
                nidx = kg * BLK
                msgs2 = msg_pool.tile([BLK, MAX_G_CHUNK * h2], BF16, tag="m2")
                out_ap = msgs2[:, : kg * h2].rearrange("p (t w) -> p t w", w=h2)
                nc.gpsimd.dma_gather(
                    out_ap,
                    y2_pairs,
                    idx16p_sb[:, 8 * g0 : 8 * g1],
                    nidx,
                    nidx,
                    h2,
                    queue_num=next_q(),
                    single_packet=False,
                )
                for g in range(g0, g1):
                    b = blk_of_g[g]
                    if b not in psum_of:
                        psum_of[b] = agg_psum.tile([BLK, BLK], F32, tag="agg", name=f"aggc{b}")
                    gl = (g - g0) * h2
                    ohp = oh_pool.tile([BLK, 2 * BLK], BF16, tag="ohf")
                    nc.vector.tensor_scalar(
                        out=ohp[:],
                        in0=iota2_h[:],
                        scalar1=dstp_sb[:, g : g + 1],
                        scalar2=w_sb[:, g : g + 1],
                        op0=mybir.AluOpType.is_equal,
                        op1=mybir.AluOpType.mult,
                    )
                    nc.tensor.matmul(
                        out=psum_of[b][:, :hid],
                        lhsT=ohp[:, 0:BLK],
                        rhs=msgs2[:, gl : gl + hid],
                        start=(g == first_g[b]),
                        stop=False,
                    )
                    nc.tensor.matmul(
                        out=psum_of[b][:, :hid],
                        lhsT=ohp[:, BLK : 2 * BLK],
                        rhs=msgs2[:, gl + hid : gl + h2],
                        start=False,
                        stop=False,
                    )
                    if g == last_g[b]:
                        tail_c(b)

    nc.finalize()
    return nc


def run(feat, state, src, dst, edge_weight, Wzr, bzr, Wc, bc, trace=False):
    """Build + run on 8 cores; returns (new_state, BassKernelResults)."""
    n_nodes, hid = feat.shape
    n_cores = N_CORES
    shard = n_nodes // n_cores

    idx16, idx16p, dst_t, dstp_t, w_t, plan = _prep_edges(
        dst, src, edge_weight, n_nodes, n_cores
    )
    import ml_dtypes

    pos = plan["pos"]
    # global permutation: node (p, l) lives at row p*shard + pos[p, l]
    inv = np.empty((n_cores, shard), np.int64)
    for p in range(n_cores):
        inv[p, pos[p]] = np.arange(shard)
    x1 = np.concatenate([feat, state], axis=1)
    x1p = np.empty_like(x1)
    for p in range(n_cores):
        x1p[p * shard : (p + 1) * shard] = x1[p * shard : (p + 1) * shard][inv[p]]
    x1b = np.ascontiguousarray(x1p.astype(ml_dtypes.bfloat16))

    nc = _build(n_nodes, hid, plan, n_cores)

    in_maps = []
    for p in range(n_cores):
        in_maps.append(
            {
                "x1b": x1b,
                "feat_s": np.ascontiguousarray(
                    feat[p * shard : (p + 1) * shard][inv[p]]
                ),
                "state_s": np.ascontiguousarray(
                    state[p * shard : (p + 1) * shard][inv[p]]
                ),
                "idx16": np.ascontiguousarray(idx16[p]),
                "idx16p": np.ascontiguousarray(idx16p[p]),
                "dst_t": np.ascontiguousarray(dst_t[p]),
                "dstp_t": np.ascontiguousarray(dstp_t[p]),
                "w_t": np.ascontiguousarray(w_t[p]),
                "wzr": np.ascontiguousarray(Wzr, dtype=np.float32),
                "bzr": np.ascontiguousarray(bzr.reshape(1, -1), dtype=np.float32),
                "wc": np.ascontiguousarray(Wc, dtype=np.float32),
                "bc": np.ascontiguousarray(bc.reshape(1, -1), dtype=np.float32),
            }
        )

    res = run_bass_kernel_spmd(
        nc, in_maps, core_ids=list(range(n_cores)), trace=trace
    )
    shards = [res.results[p]["out"][pos[p]] for p in range(n_cores)]
    return np.concatenate(shards, axis=0), res


def kernel(feat, state, src, dst, edge_weight, Wzr, bzr, Wc, bc):
    out, _ = run(feat, state, src, dst, edge_weight, Wzr, bzr, Wc, bc, trace=False)
    return out



# revision 11
# speedup vs baseline: 1.0957x; 1.0957x over previous
"""DCGRU cell on 8 Trainium2 NeuronCores.

Strategy (dst-sharded, rs-recompute pass 2, chunked fp8 AllGather):
  - Nodes are sharded into 8 contiguous ranges (one per core); within a core
    nodes are dealt into 49 blocks of 128 by in-degree (load balance). Edges
    live on the core owning dst.
  - Pass 1 gathers x1 = [feat, state] rows (bf16, 256B) from the replicated
    x1b table (lo/hi halves for int16 indices) and segment-sums them via
    one-hot matmuls into transposed PSUM agg [128 dims, 128 dst]; the agg is
    persisted in SBUF (its feat half is reused by pass 2, since
    agg2 = A[feat] @ Wc_top + A[rs] @ Wc_bot).
  - Tail A: zr = sigmoid(aggT.T @ Wzr + bzr); z^T persisted; rs = r * state
    stored to DRAM in fp8. Every ~12 blocks one AllGather chunk ships rs to
    all cores (4 chunks, each its own DRAM tensor so the chunk collectives
    and the pass-2 gathers that consume them pipeline independently).
  - Pass 2 groups edges by (dst block, src chunk). For each chunk class, as
    soon as its collective lands, single-row rs values are gathered with
    elem_size=256B/elem_step=64B (row + 3 ignored neighbor rows), upconverted
    fp8->bf16, and accumulated via one-hot matmuls into [64, 128] PSUM, then
    added into an SBUF accumulator accT. After a block's last class:
    c^T = tanh(Wc_top^T @ A[f]^T + Wc_bot^T @ accT + bc); new_state^T =
    z^T * (state^T - c^T) + c^T, stored transposed (512B lines, no penalty).
"""

import numpy as np

import concourse.bass as bass
import concourse.bacc as bacc
import concourse.mybir as mybir
import concourse.tile as tile
from concourse.bass_utils import run_bass_kernel_spmd
from concourse.library_config import mlp
from concourse.masks import make_identity

N_NODES = 50000
N_EDGES = 640000
HID = 64
N_CORES = 8
BLK = 128            # dst nodes per block (= PSUM partition dim)
SB_BLOCKS = 4        # dst blocks per super-block (pass-1 gather scope)
MAX_G1 = 26          # cap on groups per dma_gather instruction (pass 1)
MAX_G2 = 26          # cap on groups per dma_gather instruction (pass 2)
CH_SPLIT = [0, 12, 24, 36, 49]   # chunk boundaries in blocks (4 chunks)
N_CH = 4

F32 = mybir.dt.float32
BF16 = mybir.dt.bfloat16
FP8 = mybir.dt.float8e4
I16 = mybir.dt.int16

RS_DT = BF16         # rs table dtype (pair rows: 2 nodes x 64 dims = 256B)


def _ceil16(x):
    return max(16, ((int(x) + 15) // 16) * 16)


def _prep_edges(dst, src, edge_weight, n_nodes, n_cores):
    """Partition edges by dst core/block; build pass-1 (src half) and pass-2
    (src chunk) group tables. Returns (tables, plan)."""
    shard = n_nodes // n_cores
    nblk = (shard + BLK - 1) // BLK
    split = n_nodes // 2
    e = len(dst)

    ch_rows = [(CH_SPLIT[c + 1] - CH_SPLIT[c]) * BLK for c in range(N_CH)]
    ch_rows[-1] = shard - CH_SPLIT[N_CH - 1] * BLK  # last chunk partial block
    ch_of_block = np.zeros(nblk, np.int64)
    for c in range(N_CH):
        ch_of_block[CH_SPLIT[c]:CH_SPLIT[c + 1]] = c

    dsts = dst.astype(np.int64)
    srcs = src.astype(np.int64)
    owner = dsts // shard
    local = dsts - owner * shard

    # Balance in-degree across blocks: per core, deal nodes (sorted by
    # in-degree, desc) round-robin over blocks.
    deg = np.zeros(n_nodes, np.int64)
    np.add.at(deg, dsts, 1)
    pos = np.empty((n_cores, shard), np.int64)
    blk_fill = np.empty(nblk, np.int64)
    cap = np.full(nblk, BLK, np.int64)
    cap[nblk - 1] = shard - (nblk - 1) * BLK
    for p in range(n_cores):
        nodes = np.argsort(-deg[p * shard : (p + 1) * shard], kind="stable")
        blk_fill[:] = 0
        bi = 0
        for n in nodes:
            while blk_fill[bi % nblk] >= cap[bi % nblk]:
                bi += 1
            b = bi % nblk
            pos[p, n] = b * BLK + blk_fill[b]
            blk_fill[b] += 1
            bi += 1
    newloc = pos[owner, local]
    b_of = newloc // BLK
    dloc = (newloc % BLK).astype(np.float64)

    src_owner = srcs // shard
    src_l = pos[src_owner, srcs % shard]          # permuted local row of src
    src_pos = src_owner * shard + src_l           # global row in x1b
    cls1 = (src_pos >= split).astype(np.int64)
    src_local1 = src_pos - cls1 * split           # pass-1 idx (< 25000)

    src_ch = ch_of_block[src_l // BLK]            # pass-2 class (src chunk)
    row2 = src_owner * np.array(ch_rows)[src_ch] + (
        src_l - np.array(CH_SPLIT)[src_ch] * BLK
    )                                             # pass-2 idx within chunk tbl

    w64 = edge_weight.astype(np.float64)

    def build_pass(cls, idxval, n_cls, unit_order, force_units, par=None):
        """Group edges into (block, cls) units following unit_order.

        Returns idx16 [cores,16,8*ng], dst_t, w_t [cores,128,ng] (f64),
        chunks [(g0,g1,cls,nidx)], blk_of_g, first_g/last_g dicts keyed
        (b, cls), ngroups.
        """
        cnt = np.zeros((n_cores, nblk, n_cls), np.int64)
        np.add.at(cnt, (owner, b_of, cls), 1)
        unit_max = cnt.max(axis=0)                   # [nblk, n_cls]
        gp = -(-unit_max // BLK)                     # groups per unit
        for (b, c) in force_units:
            gp[b, c] = max(gp[b, c], 1)

        unit_sizes = np.array([gp[b, c] for (b, c) in unit_order], np.int64)
        unit_off = np.concatenate([[0], np.cumsum(unit_sizes)])
        ngroups = int(unit_off[-1])
        unit_idx = {bc: i for i, bc in enumerate(unit_order)}

        blk_of_g = np.zeros(ngroups, np.int64)
        cls_of_u = {}
        for i, (b, c) in enumerate(unit_order):
            blk_of_g[unit_off[i] : unit_off[i + 1]] = b
            cls_of_u[(b, c)] = i
        first_g = {}
        last_g = {}
        for i, (b, c) in enumerate(unit_order):
            if gp[b, c] > 0:
                first_g[(b, c)] = int(unit_off[i])
                last_g[(b, c)] = int(unit_off[i + 1]) - 1

        # number of real idxs in each unit (rounded up to 16)
        unit_n16 = np.array(
            [_ceil16(unit_max[b, c]) if gp[b, c] > 0 else 0 for (b, c) in unit_order],
            np.int64,
        )
        # clamp to group capacity
        unit_n16 = np.minimum(unit_n16, unit_sizes * BLK)

        # gather chunks: runs of same class in unit_order, capped; slots past
        # the last real edge of a chunk's tail unit are trimmed off num_idxs
        # (un-gathered slots stay stale in SBUF; their one-hot weights are 0).
        maxg = MAX_G1 if n_cls == 2 else MAX_G2
        chunks = []
        i = 0
        while i < len(unit_order):
            c = unit_order[i][1]
            j = i
            while j < len(unit_order) and unit_order[j][1] == c:
                j += 1
            g_run1 = int(unit_off[j]) if j < len(unit_order) else ngroups
            s = int(unit_off[i])
            while s < g_run1:
                t = min(s + maxg, g_run1)
                u = int(np.searchsorted(unit_off, t - 1, side="right") - 1)
                if t == unit_off[u + 1]:
                    # chunk ends at unit u's end: drop u's tail padding
                    lu0 = max(int(unit_off[u]), s)
                    done_before = (lu0 - int(unit_off[u])) * BLK
                    tail = int(unit_n16[u]) - done_before
                    tail = max(16, min(tail, (t - lu0) * BLK))
                    nidx = (lu0 - s) * BLK + tail
                else:
                    nidx = (t - s) * BLK
                chunks.append((int(s), int(t), int(c), int(_ceil16(nidx))))
                s = t
            i = j

        # slot assignment: rank within (core, unit)
        ukey = np.array([unit_idx[(b, c)] for b, c in zip(b_of, cls)], np.int64)
        ck = owner * len(unit_order) + ukey
        order2 = np.argsort(ck, kind="stable")
        ck_s = ck[order2]
        owner_s = owner[order2]
        idx_s = idxval[order2]
        ukey_s = ukey[order2]
        dloc_s = dloc[order2]
        ws_s = w64[order2]
        bucket_start = np.searchsorted(ck_s, np.arange(n_cores * len(unit_order)))
        rank = np.arange(e) - bucket_start[ck_s]
        g_global = unit_off[ukey_s] + rank // BLK
        lane = rank % BLK

        idx16 = np.zeros((n_cores, 16, 8 * ngroups), np.int16)
        dst_t = np.zeros((n_cores, BLK, ngroups), np.float64)
        w_t = np.zeros((n_cores, BLK, ngroups), np.float64)
        idx16[owner_s, lane % 16, 8 * g_global + lane // 16] = idx_s.astype(np.int16)
        if par is None:
            dst_t[owner_s, lane, g_global] = dloc_s
        else:
            dst_t[owner_s, lane, g_global] = dloc_s + BLK * par[order2]
        w_t[owner_s, lane, g_global] = ws_s
        return {
            "idx16": idx16,
            "dst_t": dst_t,
            "w_t": w_t,
            "chunks": chunks,
            "blk_of_g": [int(x) for x in blk_of_g],
            "first_g": first_g,
            "last_g": last_g,
            "ngroups": ngroups,
            "gp": gp,
        }

    # ---- pass 1: units (sb, half, block) ----
    unit_order1 = []
    for sb0 in range(0, nblk, SB_BLOCKS):
        sbb = range(sb0, min(sb0 + SB_BLOCKS, nblk))
        for c in range(2):
            for b in sbb:
                unit_order1.append((b, c))
    force1 = [(b, 0) for b in range(nblk)]
    p1 = build_pass(cls1, src_local1, 2, unit_order1, force1)

    # ---- pass 2: units (chunk-class, sb, block), class-major ----
    unit_order2 = []
    for c in range(N_CH):
        for sb0 in range(0, nblk, SB_BLOCKS):
            sbb = range(sb0, min(sb0 + SB_BLOCKS, nblk))
            for b in sbb:
                unit_order2.append((b, c))
    force2 = [(b, 0) for b in range(nblk)]
    p2 = build_pass(src_ch, row2 // 2, N_CH, unit_order2, force2, par=(row2 % 2))

    last_cls = np.zeros(nblk, np.int64)
    for b in range(nblk):
        for c in range(N_CH):
            if p2["gp"][b, c] > 0:
                last_cls[b] = c

    plan = {
        "p1": p1,
        "p2": p2,
        "nblk": nblk,
        "shard": shard,
        "ch_rows": ch_rows,
        "last_cls": [int(x) for x in last_cls],
        "pos": pos,
    }
    return plan


def _build(n_nodes, hid, plan, n_cores, n_queues=4):
    """Build the SPMD Bass program from the edge plan."""
    shard = plan["shard"]
    nblk = plan["nblk"]
    p1, p2 = plan["p1"], plan["p2"]
    ch_rows = plan["ch_rows"]
    last_cls = plan["last_cls"]
    split = n_nodes // 2
    h2 = 2 * hid
    ng1, ng2 = p1["ngroups"], p2["ngroups"]
    npad = nblk * BLK  # 6272

    nc = bacc.Bacc(None, num_devices=n_cores, num_swdge_queues=n_queues)

    x1b = nc.dram_tensor("x1b", [n_nodes, h2], BF16, kind="ExternalInput")
    st_d = nc.dram_tensor("st_d", [npad, hid], BF16, kind="ExternalInput")
    stT_d = nc.dram_tensor("stT_d", [hid, npad], BF16, kind="ExternalInput")
    idx1_d = nc.dram_tensor("idx1", [BLK, 8 * ng1], I16, kind="ExternalInput")
    idx2_d = nc.dram_tensor("idx2", [BLK, 8 * ng2], I16, kind="ExternalInput")
    dst1_d = nc.dram_tensor("dst1", [BLK, ng1], F32, kind="ExternalInput")
    w1_d = nc.dram_tensor("w1", [BLK, ng1], F32, kind="ExternalInput")
    dst2_d = nc.dram_tensor("dst2", [BLK, ng2], F32, kind="ExternalInput")
    w2_d = nc.dram_tensor("w2", [BLK, ng2], F32, kind="ExternalInput")
    wzr = nc.dram_tensor("wzr", [h2, h2], F32, kind="ExternalInput")
    bzr = nc.dram_tensor("bzr", [1, h2], F32, kind="ExternalInput")
    wc = nc.dram_tensor("wc", [h2, hid], F32, kind="ExternalInput")
    bc = nc.dram_tensor("bc", [1, hid], F32, kind="ExternalInput")
    outT = nc.dram_tensor("outT", [hid, npad], F32, kind="ExternalOutput")

    rs_sh = [
        nc.dram_tensor(f"rs_sh{c}", [ch_rows[c], hid], RS_DT, kind="Internal")
        for c in range(N_CH)
    ]
    rs_full = [
        nc.dram_tensor(
            f"rs_full{c}",
            [n_cores * ch_rows[c] + 2, hid],
            RS_DT,
            kind="Internal",
            addr_space="Shared",
        )
        for c in range(N_CH)
    ]

    qn = [0]

    def next_q():
        q = qn[0]
        qn[0] = (qn[0] + 1) % n_queues
        return q

    def rows_of(b):
        return BLK if b < nblk - 1 else shard - (nblk - 1) * BLK

    with tile.TileContext(nc) as tc:
        with (
            tc.tile_pool(name="const", bufs=1) as const_pool,
            tc.tile_pool(name="store", bufs=1) as store_pool,
            tc.tile_pool(name="msg", bufs=3) as msg_pool,
            tc.tile_pool(name="oh", bufs=6) as oh_pool,
            tc.tile_pool(name="blk", bufs=6) as blk_pool,
            tc.tile_pool(name="agg_ps", bufs=4, space="PSUM") as agg_psum,
            tc.tile_pool(name="agg2_ps", bufs=2, space="PSUM") as agg2_psum,
            tc.tile_pool(name="mm_ps", bufs=2, space="PSUM") as mm_psum,
        ):
            nc.gpsimd.load_library(mlp)
            # ---- constants ----
            iota_i = const_pool.tile([BLK, BLK], mybir.dt.int32)
            nc.gpsimd.iota(iota_i[:], pattern=[[1, BLK]], base=0, channel_multiplier=0)
            iota_h = const_pool.tile([BLK, BLK], BF16)
            nc.vector.tensor_copy(iota_h[:], iota_i[:])
            iota2_i = const_pool.tile([BLK, 2 * BLK], mybir.dt.int32)
            nc.gpsimd.iota(
                iota2_i[:], pattern=[[1, 2 * BLK]], base=0, channel_multiplier=0
            )
            iota2_h = const_pool.tile([BLK, 2 * BLK], BF16)
            nc.vector.tensor_copy(iota2_h[:], iota2_i[:])
            identity = const_pool.tile([BLK, BLK], F32)
            make_identity(nc, identity[:])
            ones1 = const_pool.tile([1, BLK], F32)
            nc.vector.memset(ones1[:], 1.0)
            wzr_sb = const_pool.tile([h2, h2], F32)
            nc.sync.dma_start(out=wzr_sb[:], in_=wzr[:, :])
            bzr_sb = const_pool.tile([1, h2], F32)
            nc.sync.dma_start(out=bzr_sb[:], in_=bzr[:, :])
            wctop_sb = const_pool.tile([hid, hid], F32)
            nc.sync.dma_start(out=wctop_sb[:], in_=wc[0:hid, :])
            wcbot_sb = const_pool.tile([hid, hid], F32)
            nc.sync.dma_start(out=wcbot_sb[:], in_=wc[hid:h2, :])
            bc_sb = const_pool.tile([1, hid], F32)
            nc.sync.dma_start(out=bc_sb[:], in_=bc[:, :])

            # ---- persistent tables / stores ----
            idx1_sb = store_pool.tile([BLK, 8 * ng1], I16)
            nc.sync.dma_start(out=idx1_sb[:], in_=idx1_d[:, :])
            idx2_sb = store_pool.tile([BLK, 8 * ng2], I16)
            nc.sync.dma_start(out=idx2_sb[:], in_=idx2_d[:, :])
            dst1_sb = store_pool.tile([BLK, ng1], F32)
            nc.sync.dma_start(out=dst1_sb[:], in_=dst1_d[:, :])
            w1_sb = store_pool.tile([BLK, ng1], F32)
            nc.sync.dma_start(out=w1_sb[:], in_=w1_d[:, :])
            dst2_sb = store_pool.tile([BLK, ng2], F32)
            nc.sync.dma_start(out=dst2_sb[:], in_=dst2_d[:, :])
            w2_sb = store_pool.tile([BLK, ng2], F32)
            nc.sync.dma_start(out=w2_sb[:], in_=w2_d[:, :])

            st_store = store_pool.tile([BLK, nblk * hid], BF16)
            nc.vector.memset(st_store[:], 0.0)
            for b0 in range(0, nblk, 7):
                b1 = min(b0 + 7, nblk)
                nc.sync.dma_start(
                    out=st_store[:, b0 * hid : b1 * hid].rearrange(
                        "l (b h) -> l b h", h=hid
                    ),
                    in_=st_d[b0 * BLK : b1 * BLK, :].rearrange(
                        "(b l) h -> l b h", l=BLK
                    ),
                )
            stT_store = store_pool.tile([hid, npad], BF16)
            nc.sync.dma_start(out=stT_store[:], in_=stT_d[:, :])

            aggT_store = store_pool.tile([h2, npad], F32)
            accT = store_pool.tile([hid, npad], F32)
            zT_store = store_pool.tile([hid, npad], F32)

            # pre-warm msg rings so trimmed gather slots read stale-but-finite
            for i in range(3):
                t = msg_pool.tile([BLK, MAX_G1 * h2], BF16, tag="m1", name=f"m1w{i}")
                nc.vector.memset(t[:], 0.0)
                t2 = msg_pool.tile([BLK, MAX_G2 * h2], BF16, tag="m2", name=f"m2w{i}")
                nc.vector.memset(t2[:], 0.0)

            psum_of = {}

            # ============== Phase A: pass-1 aggregation ===============
            def tail_a(b):
                R = rows_of(b)
                nc.vector.tensor_copy(
                    aggT_store[:, b * BLK : (b + 1) * BLK], psum_of.pop(b)[:]
                )
                zr_ps = mm_psum.tile([BLK, h2], F32, tag="mm")
                nc.tensor.matmul(
                    zr_ps[:],
                    lhsT=aggT_store[:, b * BLK : (b + 1) * BLK],
                    rhs=wzr_sb[:],
                    start=True,
                    stop=False,
                )
                nc.tensor.matmul(
                    zr_ps[:], lhsT=ones1[:], rhs=bzr_sb[:], start=False, stop=True
                )
                zr_sb = blk_pool.tile([BLK, h2], F32, tag="zr")
                nc.scalar.activation(
                    zr_sb[:], zr_ps[:], mybir.ActivationFunctionType.Sigmoid
                )
                ztp = mm_psum.tile([hid, BLK], F32, tag="mm")
                nc.tensor.transpose(
                    out=ztp[:], in_=zr_sb[:, 0:hid], identity=identity[:]
                )
                nc.vector.tensor_copy(zT_store[:, b * BLK : (b + 1) * BLK], ztp[:])
                rs = blk_pool.tile([BLK, hid], RS_DT, tag="rs")
                nc.vector.tensor_tensor(
                    out=rs[:],
                    in0=zr_sb[:, hid:h2],
                    in1=st_store[:, b * hid : (b + 1) * hid],
                    op=mybir.AluOpType.mult,
                )
                ch = 0
                while b >= CH_SPLIT[ch + 1]:
                    ch += 1
                r0 = (b - CH_SPLIT[ch]) * BLK
                nc.sync.dma_start(out=rs_sh[ch][r0 : r0 + R, :], in_=rs[:R, :])
                if b == CH_SPLIT[ch + 1] - 1:
                    nc.gpsimd.collective_compute(
                        "AllGather",
                        mybir.AluOpType.bypass,
                        replica_groups=[list(range(n_cores))],
                        ins=[rs_sh[ch][:, :]],
                        outs=[rs_full[ch][0 : n_cores * ch_rows[ch], :]],
                    )

            blk1 = p1["blk_of_g"]
            f1 = {}
            l1 = {}
            for b in range(nblk):
                gs = [
                    p1["first_g"].get((b, c)) for c in range(2) if (b, c) in p1["first_g"]
                ]
                ge = [
                    p1["last_g"].get((b, c)) for c in range(2) if (b, c) in p1["last_g"]
                ]
                f1[b] = min(gs)
                l1[b] = max(ge)

            for g0, g1, c, nidx in p1["chunks"]:
                kg = g1 - g0
                tbl = x1b[0:split, :] if c == 0 else x1b[split:n_nodes, :]
                msgs = msg_pool.tile([BLK, MAX_G1 * h2], BF16, tag="m1")
                out_ap = msgs[:, : kg * h2].rearrange("p (t w) -> p t w", w=h2)
                nc.gpsimd.dma_gather(
                    out_ap,
                    tbl,
                    idx1_sb[:, 8 * g0 : 8 * g0 + nidx // 16],
                    nidx,
                    nidx,
                    h2,
                    queue_num=next_q(),
                    single_packet=False,
                )
                for g in range(g0, g1):
                    b = blk1[g]
                    if b not in psum_of:
                        psum_of[b] = agg_psum.tile(
                            [h2, BLK], F32, tag="agg", name=f"agga{b}"
                        )
                    oh = oh_pool.tile([BLK, BLK], BF16, tag="oh")
                    nc.vector.tensor_scalar(
                        out=oh[:],
                        in0=iota_h[:],
                        scalar1=dst1_sb[:, g : g + 1],
                        scalar2=w1_sb[:, g : g + 1],
                        op0=mybir.AluOpType.is_equal,
                        op1=mybir.AluOpType.mult,
                    )
                    gl = (g - g0) * h2
                    nc.tensor.matmul(
                        out=psum_of[b][:],
                        lhsT=msgs[:, gl : gl + h2],
                        rhs=oh[:],
                        start=(g == f1[b]),
                        stop=(g == l1[b]),
                    )
                    if g == l1[b]:
                        tail_a(b)

            # ============== Phase C: pass-2 aggregation + output ===========
            def tail_c(b):
                R = rows_of(b)
                cps = mm_psum.tile([hid, BLK], F32, tag="mm")
                nc.tensor.matmul(
                    cps[:],
                    lhsT=wctop_sb[:],
                    rhs=aggT_store[0:hid, b * BLK : (b + 1) * BLK],
                    start=True,
                    stop=False,
                )
                nc.tensor.matmul(
                    cps[:],
                    lhsT=wcbot_sb[:],
                    rhs=accT[:, b * BLK : (b + 1) * BLK],
                    start=False,
                    stop=False,
                )
                nc.tensor.matmul(
                    cps[:], lhsT=bc_sb[:], rhs=ones1[:], start=False, stop=True
                )
                cT = blk_pool.tile([hid, BLK], F32, tag="cT")
                nc.scalar.activation(
                    cT[:], cps[:], mybir.ActivationFunctionType.Tanh
                )
                t1 = blk_pool.tile([hid, BLK], F32, tag="t1")
                nc.vector.tensor_tensor(
                    out=t1[:],
                    in0=stT_store[:, b * BLK : (b + 1) * BLK],
                    in1=cT[:],
                    op=mybir.AluOpType.subtract,
                )
                t2 = blk_pool.tile([hid, BLK], F32, tag="t2")
                nc.vector.tensor_tensor(
                    out=t2[:],
                    in0=t1[:],
                    in1=zT_store[:, b * BLK : (b + 1) * BLK],
                    op=mybir.AluOpType.mult,
                )
                nsT = blk_pool.tile([hid, BLK], F32, tag="nsT")
                nc.vector.tensor_tensor(
                    out=nsT[:], in0=t2[:], in1=cT[:], op=mybir.AluOpType.add
                )
                nc.sync.dma_start(
                    out=outT[:, b * BLK : b * BLK + R], in_=nsT[:, :R]
                )

            blk2 = p2["blk_of_g"]
            first2, last2 = p2["first_g"], p2["last_g"]
            psum2 = {}

            for g0, g1, c2, nidx in p2["chunks"]:
                kg = g1 - g0
                msgs2 = msg_pool.tile([BLK, MAX_G2 * h2], BF16, tag="m2")
                out_ap = msgs2[:, : kg * h2].rearrange("p (t w) -> p t w", w=h2)
                nc.gpsimd.dma_gather(
                    out_ap,
                    rs_full[c2][:, :].rearrange("(a b) h -> a (b h)", b=2),
                    idx2_sb[:, 8 * g0 : 8 * g0 + nidx // 16],
                    nidx,
                    nidx,
                    h2,
                    queue_num=next_q(),
                    single_packet=False,
                )
                for g in range(g0, g1):
                    b = blk2[g]
                    key = (b, c2)
                    if key not in psum2:
                        psum2[key] = agg2_psum.tile(
                            [hid, BLK], F32, tag="agg2", name=f"aggc{b}_{c2}"
                        )
                    oh = oh_pool.tile([BLK, 2 * BLK], BF16, tag="oh2")
                    nc.vector.tensor_scalar(
                        out=oh[:],
                        in0=iota2_h[:],
                        scalar1=dst2_sb[:, g : g + 1],
                        scalar2=w2_sb[:, g : g + 1],
                        op0=mybir.AluOpType.is_equal,
                        op1=mybir.AluOpType.mult,
                    )
                    gl = (g - g0) * h2
                    nc.tensor.matmul(
                        out=psum2[key][:],
                        lhsT=msgs2[:, gl : gl + hid],
                        rhs=oh[:, 0:BLK],
                        start=(g == first2[key]),
                        stop=False,
                    )
                    nc.tensor.matmul(
                        out=psum2[key][:],
                        lhsT=msgs2[:, gl + hid : gl + h2],
                        rhs=oh[:, BLK : 2 * BLK],
                        start=False,
                        stop=(g == last2[key]),
                    )
                    if g == last2[key]:
                        ps = psum2.pop(key)
                        if c2 == 0:
                            nc.vector.tensor_copy(
                                accT[:, b * BLK : (b + 1) * BLK], ps[:]
                            )
                        else:
                            nc.vector.tensor_tensor(
                                out=accT[:, b * BLK : (b + 1) * BLK],
                                in0=ps[:],
                                in1=accT[:, b * BLK : (b + 1) * BLK],
                                op=mybir.AluOpType.add,
                            )
                        if c2 == last_cls[b]:
                            tail_c(b)

    nc.finalize()
    return nc


def run(feat, state, src, dst, edge_weight, Wzr, bzr, Wc, bc, trace=False):
    """Build + run on 8 cores; returns (new_state, BassKernelResults)."""
    import ml_dtypes

    n_nodes, hid = feat.shape
    n_cores = N_CORES
    shard = n_nodes // n_cores

    plan = _prep_edges(dst, src, edge_weight, n_nodes, n_cores)
    pos = plan["pos"]
    nblk = plan["nblk"]
    npad = nblk * BLK
    p1, p2 = plan["p1"], plan["p2"]

    # global permutation: node (p, l) lives at row p*shard + pos[p, l]
    inv = np.empty((n_cores, shard), np.int64)
    for p in range(n_cores):
        inv[p, pos[p]] = np.arange(shard)
    x1 = np.concatenate([feat, state], axis=1)
    x1p = np.empty_like(x1)
    for p in range(n_cores):
        x1p[p * shard : (p + 1) * shard] = x1[p * shard : (p + 1) * shard][inv[p]]
    x1b = np.ascontiguousarray(x1p.astype(ml_dtypes.bfloat16))

    nc = _build(n_nodes, hid, plan, n_cores)

    in_maps = []
    for p in range(n_cores):
        st_p = state[p * shard : (p + 1) * shard][inv[p]].astype(ml_dtypes.bfloat16)
        st_pad = np.zeros((npad, hid), ml_dtypes.bfloat16)
        st_pad[:shard] = st_p
        stT_pad = np.zeros((hid, npad), ml_dtypes.bfloat16)
        stT_pad[:, :shard] = st_p.T
        in_maps.append(
            {
                "x1b": x1b,
                "st_d": np.ascontiguousarray(st_pad),
                "stT_d": np.ascontiguousarray(stT_pad),
                "idx1": np.ascontiguousarray(np.tile(p1["idx16"][p], (8, 1))),
                "idx2": np.ascontiguousarray(np.tile(p2["idx16"][p], (8, 1))),
                "dst1": np.ascontiguousarray(p1["dst_t"][p].astype(np.float32)),
                "w1": np.ascontiguousarray(p1["w_t"][p].astype(np.float32)),
                "dst2": np.ascontiguousarray(p2["dst_t"][p].astype(np.float32)),
                "w2": np.ascontiguousarray(p2["w_t"][p].astype(np.float32)),
                "wzr": np.ascontiguousarray(Wzr, dtype=np.float32),
                "bzr": np.ascontiguousarray(bzr.reshape(1, -1), dtype=np.float32),
                "wc": np.ascontiguousarray(Wc, dtype=np.float32),
                "bc": np.ascontiguousarray(bc.reshape(1, -1), dtype=np.float32),
            }
        )

    res = run_bass_kernel_spmd(
        nc, in_maps, core_ids=list(range(n_cores)), trace=trace
    )
    shards = [
        res.results[p]["outT"][:, :shard].T[pos[p]] for p in range(n_cores)
    ]
    return np.concatenate(shards, axis=0), res


def kernel(feat, state, src, dst, edge_weight, Wzr, bzr, Wc, bc):
    out, _ = run(feat, state, src, dst, edge_weight, Wzr, bzr, Wc, bc, trace=False)
    return out
